# revision 13
# baseline (speedup 1.0000x reference)
"""ART reconstruction kernel for Trainium2 (8 NeuronCores).

Structure exploited: the reference's forward projection indexes the
*flattened* image with detector indices clamped to [0, 255], so it only
ever reads row 0 of the image. The 270-step recurrence therefore acts on
a tiny (B, 256) row-state on the host, and the final image is a
backprojection of per-angle residual sums Rs (A, B, D):

    image[b, i, j] = sum_a Rs[a, b, idx[a, i, j]]        (Rs pre-scaled)

Device work (batch-parallel, 2 batch elements per core): for each angle
the gather over detector bins is a matmul against device-built 0/1 step
matrices. idx[a, i, :] is monotone in j, so

    [idx[a,i,j] >= k] = [iota_a(j) >= t[a,k,i]],  t = #{j: idx < k}

with iota_a = j (cos>=0) or 255-j (cos<0), and the telescoped sum
    sum_k DRs[a,b,k] * S_a[k, (j,i)] = Rs[a, b, idx]
runs on the PE with DRs split hi+lo fp16 to keep f32-level accuracy.

The step-threshold table (constant geometry) stays device-resident as a
jax array across calls; per-call traffic is only the 92 KB/core DRs pack
in and the 512 KB/core image out.
"""

import numpy as np

import concourse.bass as bass
import concourse.mybir as mybir

H = W = 256
D = 256
A = 90
B = 16
ITERS = 3
NCORES = 8
KT = 2            # k tiles of 128
BPC = B // NCORES  # batches per core
NCH = 32           # j-column chunks (psum_bp = 4 banks)
JC = W // NCH      # j columns per chunk
CPIX = JC * H      # pixels per chunk
NT = A * KT        # S tiles per chunk
NMM = 512          # matmul moving free size
QN = CPIX // NMM
NSTEP = ITERS * A
INV = float(1.0 / 256.0)

_cache = {}


# ---------------------------------------------------------------- geometry
def _geometry():
    """Detector index map (A, H, W) int32 + per-angle cos sign, faithful to
    the reference (jax f32 on cpu; numpy fallback)."""
    if "idx" in _cache:
        return _cache["idx"], _cache["signs"]
    try:
        import jax
        import jax.numpy as jnp

        with jax.default_device(jax.devices("cpu")[0]):
            angles = jnp.linspace(0.0, np.pi, A)
            y, x = jnp.meshgrid(
                jnp.arange(H, dtype=jnp.float32),
                jnp.arange(W, dtype=jnp.float32),
                indexing="ij",
            )
            x_c = x - W / 2.0
            y_c = y - H / 2.0
            rot = (
                x_c[None] * jnp.cos(angles)[:, None, None]
                + y_c[None] * jnp.sin(angles)[:, None, None]
            )
            idx = (rot / (2.0 * np.pi) * D).astype(jnp.int32)
            idx = np.asarray(jnp.clip(idx, 0, D - 1))
            signs = np.asarray(jnp.cos(angles)) >= 0.0
    except Exception:
        angles = np.linspace(0.0, np.pi, A, dtype=np.float64).astype(np.float32)
        y, x = np.meshgrid(
            np.arange(H, dtype=np.float32),
            np.arange(W, dtype=np.float32),
            indexing="ij",
        )
        x_c = (x - np.float32(W / 2.0)).astype(np.float32)
        y_c = (y - np.float32(H / 2.0)).astype(np.float32)
        rot = (
            x_c[None] * np.cos(angles)[:, None, None]
            + y_c[None] * np.sin(angles)[:, None, None]
        ).astype(np.float32)
        idx = np.clip((rot / np.float32(2.0 * np.pi) * D).astype(np.int32), 0, D - 1)
        signs = np.cos(angles) >= 0.0
    _cache["idx"] = idx
    _cache["signs"] = signs
    return idx, signs


def _recurrence_consts():
    """C[a] (D,D) forward-projection count matrices + row-0 gather map."""
    if "C" in _cache:
        return _cache["C"], _cache["idx0"]
    idx, _ = _geometry()
    lin = (
        np.arange(A, dtype=np.int64)[:, None, None] * (W * D)
        + np.arange(W, dtype=np.int64)[None, None, :] * D
        + idx.astype(np.int64)
    )  # (A, H, W): bin = a*W*D + j*D + k
    counts = np.bincount(lin.ravel(), minlength=A * W * D).reshape(A, W, D)
    C = np.ascontiguousarray(counts.transpose(0, 2, 1)).astype(np.float32)
    idx0 = np.ascontiguousarray(idx[:, 0, :])
    _cache["C"] = C
    _cache["idx0"] = idx0
    return C, idx0


def _thresholds():
    """t[a, k, i] = #{j : idx[a,i,j] < k}; packed [128, A*KT*H] fp16."""
    if "thr" in _cache:
        return _cache["thr"]
    idx, _ = _geometry()
    rows = idx.reshape(A * H, W)
    off = np.arange(A * H, dtype=np.int64)[:, None] * D
    hist = np.bincount((rows + off).ravel(), minlength=A * H * D).reshape(A, H, D)
    below = np.concatenate(
        [np.zeros((A, H, 1), np.int64), np.cumsum(hist, axis=-1)[:, :, :-1]],
        axis=-1,
    )  # (A, i, k)
    thr = below.transpose(0, 2, 1)  # (A, k, i)
    pack = thr.reshape(A, KT, 128, H).transpose(2, 0, 1, 3).reshape(128, A * KT * H)
    pack = np.ascontiguousarray(pack, dtype=np.float16)
    _cache["thr"] = pack
    return pack


# ---------------------------------------------------------------- host math
def _host_residuals(sinograms):
    """Row-state recurrence; returns Rs (A, B, D) f32, scaled by 1/256."""
    C, idx0 = _recurrence_consts()
    sino = np.ascontiguousarray(np.transpose(sinograms, (1, 0, 2))).astype(
        np.float32
    )
    r = np.zeros((B, D), dtype=np.float32)
    Rs = np.zeros((A, B, D), dtype=np.float32)
    fp = np.empty((B, D), dtype=np.float32)
    res = np.empty((B, D), dtype=np.float32)
    g = np.empty((B, D), dtype=np.float32)
    inv = np.float32(1.0 / 256.0)
    for _ in range(ITERS):
        for a in range(A):
            np.dot(r, C[a], out=fp)
            np.subtract(sino[a], fp, out=res)
            Rs[a] += res
            np.take(res, idx0[a], axis=1, out=g)
            g *= inv
            r += g
    Rs *= inv
    return Rs


def _pack_drs_all(Rs):
    """(NCORES*128, A*KT*2*BPC) fp16 concat of per-core hi+lo lhsT packs."""
    if "drs_perm" not in _cache:
        _cache["drs_perm"] = (
            np.arange(A * 2 * B * D)
            .reshape(A, 2, NCORES, BPC, KT, 128)  # (a, h, c, b, kt, p)
            .transpose(2, 5, 0, 4, 1, 3)          # (c, p, a, kt, h, b)
            .ravel()
        )
    DRs = Rs.copy()
    DRs[:, :, 1:] -= Rs[:, :, :-1]
    hi = DRs.astype(np.float16)
    lo = (DRs - hi.astype(np.float32)).astype(np.float16)
    sl = np.stack([hi, lo], axis=1)  # (A, h, B, D)
    flat = sl.view(np.uint16).ravel()[_cache["drs_perm"]].view(np.float16)
    return flat.reshape(NCORES * 128, A * KT * 2 * BPC)


# ---------------------------------------------------------------- device
def _build_nc(signs):
    nc = bass.Bass()
    f16 = mybir.dt.float16
    f32 = mybir.dt.float32
    thr_d = nc.declare_dram_parameter("thr", [128, NT * H], f16, isOutput=False)
    cg_d = nc.declare_dram_parameter("cg", [128, A * 4 * 256], f16, isOutput=False)
    diff_d = nc.declare_dram_parameter("diff", [128, 2 * 256], f32, isOutput=False)
    sino_d = nc.declare_dram_parameter(
        "sino", [128, A * 2 * BPC], f32, isOutput=False
    )
    out_d = nc.declare_dram_parameter("out", [BPC, H * W], f16, isOutput=True)

    from contextlib import ExitStack

    with ExitStack() as stack:
        ec = stack.enter_context
        thr_sb = ec(nc.sbuf_tensor([128, NT * H], f16))
        sino_sb = ec(nc.sbuf_tensor([128, A * 2 * BPC], f32))
        diff_sb = ec(nc.sbuf_tensor([128, 2 * 128 * 2], f32))  # [dt, kt*128+k]
        cg_sb = ec(nc.sbuf_tensor([128, 3 * 4 * 256], f16))    # [buf, blk, f]
        rs_sb = ec(nc.sbuf_tensor([128, A * 2 * BPC], f32))    # [a, dt, b]
        rt16 = ec(nc.sbuf_tensor([128, 8], f16))               # [kt, h, b]
        r32 = ec(nc.sbuf_tensor([128, 4], f32))                # [kt, b]
        tmp32 = ec(nc.sbuf_tensor([128, 4], f32))              # [dt, b]
        tmp32b = ec(nc.sbuf_tensor([128, 4], f32))             # [kt, b]
        res16 = ec(nc.sbuf_tensor([128, 4], f16))              # [dt, b]
        drs_sb = ec(nc.sbuf_tensor([128, NT * 2 * BPC], f16))  # [a, kt, h, b]
        jfwd = ec(nc.sbuf_tensor([128, 2 * CPIX], f16))
        jrev = ec(nc.sbuf_tensor([128, 2 * CPIX], f16))
        s_sb = ec(nc.sbuf_tensor([128, 2 * CPIX], f16))
        out_sb = ec(nc.sbuf_tensor([BPC, 2 * CPIX], f16))
        psum_bp = ec(nc.psum_tensor([BPC, CPIX], f32))
        psum_fp = ec(nc.psum_tensor([128, 4], f32))            # [jt(=dt), b]
        psum_g = ec(nc.psum_tensor([128, 4], f32))             # [kt, b]
        psum_d = ec(nc.psum_tensor([128, 4], f32))             # [kt, b]
        sem_thr = ec(nc.semaphore())
        sem_sino = ec(nc.semaphore())
        sem_diff = ec(nc.semaphore())
        sem_cg = [ec(nc.semaphore(name=f"sem_cg{i}")) for i in range(3)]
        sem_fp = ec(nc.semaphore())
        sem_res = ec(nc.semaphore())
        sem_gat = ec(nc.semaphore())
        sem_rup = ec(nc.semaphore())
        sem_dmm = ec(nc.semaphore())
        sem_drs = ec(nc.semaphore())
        sem_jch = ec(nc.semaphore())
        sem_s = ec(nc.semaphore())
        sem_sc = ec(nc.semaphore())
        sem_evac = ec(nc.semaphore())
        sem_dout = ec(nc.semaphore())
        sem_vv = ec(nc.semaphore())
        sem_init = ec(nc.semaphore())
        block = ec(nc.Block())
        cg_v = cg_sb[:, :].rearrange("p (u blk f) -> p u blk f", blk=4, f=256)
        rt_v = rt16[:, :].rearrange("p (kt h b) -> p kt h b", h=2, b=BPC)
        r32_v = r32[:, :].rearrange("p (kt b) -> p kt b", b=BPC)
        sino_v = sino_sb[:, :].rearrange("p (a dt b) -> p a dt b", dt=2, b=BPC)
        rs_v = rs_sb[:, :].rearrange("p (a dt b) -> p a dt b", dt=2, b=BPC)
        res_v = res16[:, :].rearrange("p (dt b) -> p dt b", b=BPC)
        diff_v = diff_sb[:, :].rearrange("p (dt f) -> p dt f", f=256)
        drs_v = drs_sb[:, :].rearrange(
            "p (a kt h b) -> p a kt h b", kt=KT, h=2, b=BPC
        )

        @block.scalar
        def _(scalar):
            scalar.dma_start(out=sino_sb[:, :], in_=sino_d[:, :]).then_inc(
                sem_sino, 16
            )
            scalar.dma_start(out=diff_sb[:, :], in_=diff_d[:, :]).then_inc(
                sem_diff, 16
            )
            scalar.dma_start(out=thr_sb[:, :], in_=thr_d[:, :]).then_inc(
                sem_thr, 16
            )
            # backprojection psum evacuation
            for c in range(NCH):
                scalar.wait_ge(sem_sc, NT * (c + 1))
                if c >= 2:
                    scalar.wait_ge(sem_dout, 16 * c)
                src = psum_bp[:, :].rearrange("b (jj ii) -> b jj ii", ii=H)
                dst = out_sb[:, (c % 2) * CPIX:(c % 2 + 1) * CPIX].rearrange(
                    "b (ii jj) -> b jj ii", jj=JC
                )
                scalar.copy(out=dst, in_=src).then_inc(sem_evac, 1)

        @block.sync
        def _(sync):
            # stream cg packs, 3-deep ring
            for t in range(NSTEP):
                a = t % A
                if t >= 3:
                    sync.wait_ge(sem_gat, t - 2)  # step t-3 PE-consumed
                sync.dma_start(
                    out=cg_sb[:, (t % 3) * 1024:(t % 3 + 1) * 1024],
                    in_=cg_d[:, a * 1024:(a + 1) * 1024],
                ).then_inc(sem_cg[t % 3], 16)
            for c in range(NCH):
                sync.wait_ge(sem_evac, c + 1)
                src = out_sb[:, (c % 2) * CPIX:(c % 2 + 1) * CPIX].rearrange(
                    "b (i jj) -> b i jj", jj=JC
                )
                out_v = out_d[:, :].rearrange("b (i j) -> b i j", j=W)
                sync.dma_start(
                    out=out_v[:, :, c * JC:(c + 1) * JC], in_=src
                ).then_inc(sem_dout, 16)

        @block.gpsimd
        def _(gpsimd):
            gpsimd.memset(rt16[:, :], 0).then_inc(sem_init, 1)
            gpsimd.memset(r32[:, :], 0).then_inc(sem_init, 1)
            gpsimd.memset(rs_sb[:, :], 0).then_inc(sem_init, 1)
            for c in range(NCH):
                if c >= 2:
                    gpsimd.wait_ge(sem_s, NT * (c - 1))
                buf = (c % 2) * CPIX
                gpsimd.iota(
                    jfwd[:, buf:buf + CPIX].rearrange("p (jj ii) -> p jj ii", ii=H),
                    [[1, JC], [0, H]],
                    base=c * JC,
                    channel_multiplier=0,
                    allow_small_or_imprecise_dtypes=True,
                )
                gpsimd.iota(
                    jrev[:, buf:buf + CPIX].rearrange("p (jj ii) -> p jj ii", ii=H),
                    [[-1, JC], [0, H]],
                    base=(W - 1) - c * JC,
                    channel_multiplier=0,
                    allow_small_or_imprecise_dtypes=True,
                ).then_inc(sem_jch, 1)

        @block.tensor
        def _(tensor):
            # ---- recurrence ----
            for t in range(NSTEP):
                tensor.wait_ge(sem_cg[t % 3], 16 * (t // 3 + 1))
                if t == 0:
                    tensor.wait_ge(sem_init, 3)
                else:
                    tensor.wait_ge(sem_vv, 7 * t)  # r-update of step t-1
                for jt in range(2):
                    for kt in range(2):
                        for h in range(2):
                            mm = tensor.matmul(
                                psum_fp[:, jt * BPC:(jt + 1) * BPC],
                                cg_v[:, t % 3, kt, jt * 128:(jt + 1) * 128],
                                rt_v[:, kt, h, :],
                                start=(kt == 0 and h == 0),
                                stop=(kt == 1 and h == 1),
                            )
                mm.then_inc(sem_fp, 1)
                # -- gather --
                tensor.wait_ge(sem_vv, 7 * t + 3)  # res16 ready
                for kt in range(2):
                    for dt in range(2):
                        mm = tensor.matmul(
                            psum_g[:, kt * BPC:(kt + 1) * BPC],
                            cg_v[:, t % 3, 2 + dt, kt * 128:(kt + 1) * 128],
                            res_v[:, dt, :],
                            start=(dt == 0),
                            stop=(dt == 1),
                        )
                mm.then_inc(sem_gat, 1)
            # ---- DRs ----
            tensor.wait_ge(sem_diff, 16)
            tensor.wait_ge(sem_vv, 7 * NSTEP)  # recurrence VE fully done
            for a in range(A):
                if a >= 1:
                    tensor.wait_ge(sem_vv, 7 * NSTEP + 3 * a)  # split a-1 done
                for kt in range(2):
                    for dt in range(2):
                        mm = tensor.matmul(
                            psum_d[:, kt * BPC:(kt + 1) * BPC],
                            diff_v[:, dt, kt * 128:(kt + 1) * 128],
                            rs_v[:, a, dt, :],
                            start=(dt == 0),
                            stop=(dt == 1),
                        )
                mm.then_inc(sem_dmm, 1)
            # ---- backprojection ----
            gt = 0
            for c in range(NCH):
                for t in range(NT):
                    a, kt = divmod(t, KT)
                    tensor.wait_ge(sem_s, gt + 1)
                    if t == 0:
                        if c == 0:
                            tensor.wait_ge(sem_vv, 7 * NSTEP + 3 * A)  # drs done
                        else:
                            tensor.wait_ge(sem_evac, c)
                    sb = (gt % 2) * CPIX
                    for h in range(2):
                        lhsT = drs_v[:, a, kt, h, :]
                        for q in range(QN):
                            mm = tensor.matmul(
                                psum_bp[:, q * NMM:(q + 1) * NMM],
                                lhsT,
                                s_sb[:, sb + q * NMM:sb + (q + 1) * NMM],
                                start=(t == 0 and h == 0),
                                stop=(t == NT - 1 and h == 1),
                            )
                    mm.then_inc(sem_sc, 1)
                    gt += 1

        @block.vector
        def _(vector):
            # ---- recurrence partner (VE fully serialized via sem_vv) ----
            vv = [0]

            def step(ins):
                ins.then_inc(sem_vv, 1)
                vv[0] += 1
                vector.wait_ge(sem_vv, vv[0])

            vector.wait_ge(sem_sino, 16)
            for t in range(NSTEP):
                a = t % A
                vector.wait_ge(sem_fp, t + 1)
                step(vector.tensor_tensor(
                    tmp32[:, :],
                    sino_sb[:, a * 4:(a + 1) * 4],
                    psum_fp[:, :],
                    mybir.AluOpType.subtract,
                ))
                step(vector.tensor_tensor(
                    rs_sb[:, a * 4:(a + 1) * 4],
                    rs_sb[:, a * 4:(a + 1) * 4],
                    tmp32[:, :],
                    mybir.AluOpType.add,
                ))
                step(vector.tensor_copy(res16[:, :], tmp32[:, :]))
                vector.wait_ge(sem_gat, t + 1)
                step(vector.tensor_scalar(
                    tmp32b[:, :], psum_g[:, :], INV, None, mybir.AluOpType.mult
                ))
                step(vector.tensor_tensor(
                    r32[:, :], r32[:, :], tmp32b[:, :], mybir.AluOpType.add
                ))
                step(vector.tensor_copy(rt_v[:, :, 0, :], r32_v[:, :, :]))
                step(vector.tensor_tensor(
                    rt_v[:, :, 1, :],
                    r32_v[:, :, :],
                    rt_v[:, :, 0, :],
                    mybir.AluOpType.subtract,
                ))
            # ---- DRs splits ----
            for a in range(A):
                vector.wait_ge(sem_dmm, a + 1)
                step(vector.tensor_scalar(
                    tmp32b[:, :], psum_d[:, :], INV, None, mybir.AluOpType.mult
                ))
                step(vector.tensor_copy(
                    drs_v[:, a, :, 0, :],
                    tmp32b[:, :].rearrange("p (kt b) -> p kt b", b=BPC),
                ))
                step(vector.tensor_tensor(
                    drs_v[:, a, :, 1, :],
                    tmp32b[:, :].rearrange("p (kt b) -> p kt b", b=BPC),
                    drs_v[:, a, :, 0, :],
                    mybir.AluOpType.subtract,
                ))
            # ---- backprojection S builds ----
            gt = 0
            for c in range(NCH):
                vector.wait_ge(sem_jch, c + 1)
                if c == 0:
                    vector.wait_ge(sem_thr, 16)
                for t in range(NT):
                    a, kt = divmod(t, KT)
                    if gt >= 2:
                        vector.wait_ge(sem_sc, gt - 1)
                    jsrc = jfwd if signs[a] else jrev
                    buf = (c % 2) * CPIX
                    in0 = jsrc[:, buf:buf + CPIX].rearrange(
                        "p (jj ii) -> p jj ii", ii=H
                    )
                    base = (a * KT + kt) * H
                    in1 = (
                        thr_sb[:, base:base + H]
                        .unsqueeze(1)
                        .broadcast_to([128, JC, H])
                    )
                    sb = (gt % 2) * CPIX
                    outp = s_sb[:, sb:sb + CPIX].rearrange(
                        "p (jj ii) -> p jj ii", ii=H
                    )
                    vector.tensor_tensor(
                        outp, in0, in1, mybir.AluOpType.is_ge
                    ).then_inc(sem_s, 1)
                    gt += 1

    return nc


# ---------------------------------------------------------------- runner

def _make_runner():
    """Build nc + cached jitted shard_map callable + device-resident thr."""
    if "runner" in _cache:
        return _cache["runner"]

    import jax
    import jax.numpy as jnp
    from jax.experimental.shard_map import shard_map
    from jax.sharding import Mesh, NamedSharding, PartitionSpec

    from concourse.bass2jax import (
        _bass_exec_p,
        install_neuronx_cc_hook,
        partition_id_tensor,
    )

    install_neuronx_cc_hook()

    _, signs = _geometry()
    nc = _build_nc(signs)

    partition_name = nc.partition_id_tensor.name if nc.partition_id_tensor else None
    in_names, out_names, out_avals, zero_shapes = [], [], [], []
    for alloc in nc.m.functions[0].allocations:
        if not isinstance(alloc, mybir.MemoryLocationSet):
            continue
        name = alloc.memorylocations[0].name
        if alloc.kind == "ExternalInput":
            if name != partition_name:
                in_names.append(name)
        elif alloc.kind == "ExternalOutput":
            out_names.append(name)
            shape = tuple(alloc.tensor_shape)
            dtype = mybir.dt.np(alloc.dtype)
            out_avals.append(jax.core.ShapedArray(shape, dtype))
            zero_shapes.append((shape, dtype))
    n_params = len(in_names)
    n_outs = len(out_avals)
    all_in_names = in_names + out_names
    if partition_name is not None:
        all_in_names = all_in_names + [partition_name]

    def _body(*args):
        operands = list(args)
        if partition_name is not None:
            operands.append(partition_id_tensor())
        outs = _bass_exec_p.bind(
            *operands,
            out_avals=tuple(out_avals),
            in_names=tuple(all_in_names),
            out_names=tuple(out_names),
            lowering_input_output_aliases=(),
            sim_require_finite=True,
            sim_require_nnan=True,
            nc=nc,
        )
        return tuple(outs)

    devices = jax.devices()[:NCORES]
    mesh = Mesh(np.asarray(devices), ("core",))
    spec = PartitionSpec("core")
    sharded = jax.jit(
        shard_map(
            _body,
            mesh=mesh,
            in_specs=(spec,) * (n_params + n_outs),
            out_specs=(spec,) * n_outs,
            check_rep=False,
        ),
        donate_argnums=tuple(range(n_params, n_params + n_outs)),
        keep_unused=True,
    )

    def zeros_maker():
        return tuple(
            jnp.zeros((NCORES * s[0], *s[1:]), d) for (s, d) in zero_shapes
        )

    zeros_jit = jax.jit(
        zeros_maker,
        out_shardings=tuple(
            NamedSharding(mesh, spec) for _ in zero_shapes
        ),
    )

    # constant tables (geometry), device-resident once
    sh = NamedSharding(mesh, spec)

    def _replicate(arr):
        g = np.broadcast_to(arr[None], (NCORES, *arr.shape)).reshape(
            NCORES * arr.shape[0], *arr.shape[1:]
        )
        d = jax.device_put(np.ascontiguousarray(g), sh)
        jax.block_until_ready(d)
        return d

    thr_dev = _replicate(_thresholds())

    C, idx0 = _recurrence_consts()
    cg = np.zeros((128, A, 4, 256), np.float16)
    for a in range(A):
        for kt in range(2):
            cg[:, a, kt, :] = C[a, kt * 128:(kt + 1) * 128, :]  # [p, j]
        for dt in range(2):
            cg[:, a, 2 + dt, :] = (
                idx0[a][None, :] == (dt * 128 + np.arange(128))[:, None]
            ).astype(np.float16)  # [p, k]
    cg_dev = _replicate(np.ascontiguousarray(cg.reshape(128, A * 4 * 256)))

    diff = np.zeros((128, 2, 256), np.float32)
    for dt in range(2):
        dvec = dt * 128 + np.arange(128)
        diff[:, dt, :] = (dvec[:, None] == np.arange(256)[None, :]).astype(
            np.float32
        ) - (dvec[:, None] == (np.arange(256) - 1)[None, :]).astype(np.float32)
    diff_dev = _replicate(np.ascontiguousarray(diff.reshape(128, 512)))

    # per-call sinogram permutation: sp[c*128+p, (a, dt, b)] = sino[2c+b, a, dt*128+p]
    src = (
        np.arange(B * A * D)
        .reshape(NCORES, BPC, A, 2, 128)   # (c, b, a, dt, p)
        .transpose(0, 4, 2, 3, 1)          # (c, p, a, dt, b)
        .ravel()
    )
    runner = {
        "sharded": sharded,
        "zeros_jit": zeros_jit,
        "thr_dev": thr_dev,
        "cg_dev": cg_dev,
        "diff_dev": diff_dev,
        "sino_perm": src,
        "in_names": in_names,
        "out_names": out_names,
    }
    _cache["runner"] = runner
    return runner


def kernel(sinograms):
    import jax

    sinograms = np.asarray(sinograms, dtype=np.float32)
    r = _make_runner()
    sp = (
        sinograms.ravel()[r["sino_perm"]]
        .reshape(NCORES * 128, A * 2 * BPC)
    )
    arg_by_name = {
        "thr": r["thr_dev"],
        "cg": r["cg_dev"],
        "diff": r["diff_dev"],
        "sino": sp,
    }
    args = [arg_by_name[n] for n in r["in_names"]]
    # donated output buffers: reuse last call's (kernel writes every element)
    outs = r.pop("last_out", None)
    if outs is None:
        outs = r["zeros_jit"]()
    out_arrs = r["sharded"](*args, *outs)
    out_np = np.asarray(out_arrs[0])  # (NCORES*BPC, H*W) == (B, H*W)
    r["last_out"] = out_arrs

    img = out_np.astype(np.float32).reshape(B, H, W)
    np.clip(img, 0.0, img.max(), out=img)
    return img


# revision 14
# speedup vs baseline: 11.9931x; 11.9931x over previous
"""ART reconstruction kernel for Trainium2 (8 NeuronCores).

Structure exploited: the reference's forward projection indexes the
*flattened* image with detector indices clamped to [0, 255], so it only
ever reads row 0 of the image. The 270-step recurrence therefore acts on
a tiny (B, 256) row-state on the host, and the final image is a
backprojection of per-angle residual sums Rs (A, B, D):

    image[b, i, j] = sum_a Rs[a, b, idx[a, i, j]]        (Rs pre-scaled)

Device work (batch-parallel, 2 batch elements per core): for each angle
the gather over detector bins is a matmul against device-built 0/1 step
matrices. idx[a, i, :] is monotone in j, so

    [idx[a,i,j] >= k] = [iota_a(j) >= t[a,k,i]],  t = #{j: idx < k}

with iota_a = j (cos>=0) or 255-j (cos<0), and the telescoped sum
    sum_k DRs[a,b,k] * S_a[k, (j,i)] = Rs[a, b, idx]
runs on the PE with DRs split hi+lo fp16 to keep f32-level accuracy.

The step-threshold table (constant geometry) stays device-resident as a
jax array across calls; per-call traffic is only the 92 KB/core DRs pack
in and the 512 KB/core image out.
"""

import numpy as np

import concourse.bass as bass
import concourse.mybir as mybir

H = W = 256
D = 256
A = 90
B = 16
ITERS = 3
NCORES = 8
KT = 2            # k tiles of 128
BPC = B // NCORES  # batches per core
NCH = 32           # j-column chunks (psum_bp = 4 banks)
JC = W // NCH      # j columns per chunk
CPIX = JC * H      # pixels per chunk
NT = A * KT        # S tiles per chunk
NMM = 512          # matmul moving free size
QN = CPIX // NMM
NSTEP = ITERS * A
INV = float(1.0 / 256.0)

_cache = {}


# ---------------------------------------------------------------- geometry
def _geometry():
    """Detector index map (A, H, W) int32 + per-angle cos sign, faithful to
    the reference (jax f32 on cpu; numpy fallback)."""
    if "idx" in _cache:
        return _cache["idx"], _cache["signs"]
    try:
        import jax
        import jax.numpy as jnp

        with jax.default_device(jax.devices("cpu")[0]):
            angles = jnp.linspace(0.0, np.pi, A)
            y, x = jnp.meshgrid(
                jnp.arange(H, dtype=jnp.float32),
                jnp.arange(W, dtype=jnp.float32),
                indexing="ij",
            )
            x_c = x - W / 2.0
            y_c = y - H / 2.0
            rot = (
                x_c[None] * jnp.cos(angles)[:, None, None]
                + y_c[None] * jnp.sin(angles)[:, None, None]
            )
            idx = (rot / (2.0 * np.pi) * D).astype(jnp.int32)
            idx = np.asarray(jnp.clip(idx, 0, D - 1))
            signs = np.asarray(jnp.cos(angles)) >= 0.0
    except Exception:
        angles = np.linspace(0.0, np.pi, A, dtype=np.float64).astype(np.float32)
        y, x = np.meshgrid(
            np.arange(H, dtype=np.float32),
            np.arange(W, dtype=np.float32),
            indexing="ij",
        )
        x_c = (x - np.float32(W / 2.0)).astype(np.float32)
        y_c = (y - np.float32(H / 2.0)).astype(np.float32)
        rot = (
            x_c[None] * np.cos(angles)[:, None, None]
            + y_c[None] * np.sin(angles)[:, None, None]
        ).astype(np.float32)
        idx = np.clip((rot / np.float32(2.0 * np.pi) * D).astype(np.int32), 0, D - 1)
        signs = np.cos(angles) >= 0.0
    _cache["idx"] = idx
    _cache["signs"] = signs
    return idx, signs


def _recurrence_consts():
    """C[a] (D,D) forward-projection count matrices + row-0 gather map."""
    if "C" in _cache:
        return _cache["C"], _cache["idx0"]
    idx, _ = _geometry()
    lin = (
        np.arange(A, dtype=np.int64)[:, None, None] * (W * D)
        + np.arange(W, dtype=np.int64)[None, None, :] * D
        + idx.astype(np.int64)
    )  # (A, H, W): bin = a*W*D + j*D + k
    counts = np.bincount(lin.ravel(), minlength=A * W * D).reshape(A, W, D)
    C = np.ascontiguousarray(counts.transpose(0, 2, 1)).astype(np.float32)
    idx0 = np.ascontiguousarray(idx[:, 0, :])
    _cache["C"] = C
    _cache["idx0"] = idx0
    return C, idx0


def _thresholds():
    """t[a, k, i] = #{j : idx[a,i,j] < k}; packed [128, A*KT*H] fp16."""
    if "thr" in _cache:
        return _cache["thr"]
    idx, _ = _geometry()
    rows = idx.reshape(A * H, W)
    off = np.arange(A * H, dtype=np.int64)[:, None] * D
    hist = np.bincount((rows + off).ravel(), minlength=A * H * D).reshape(A, H, D)
    below = np.concatenate(
        [np.zeros((A, H, 1), np.int64), np.cumsum(hist, axis=-1)[:, :, :-1]],
        axis=-1,
    )  # (A, i, k)
    thr = below.transpose(0, 2, 1)  # (A, k, i)
    pack = thr.reshape(A, KT, 128, H).transpose(2, 0, 1, 3).reshape(128, A * KT * H)
    pack = np.ascontiguousarray(pack, dtype=np.float16)
    _cache["thr"] = pack
    return pack


# ---------------------------------------------------------------- host math
def _host_residuals(sinograms):
    """Row-state recurrence; returns Rs (A, B, D) f32, scaled by 1/256."""
    C, idx0 = _recurrence_consts()
    sino = np.ascontiguousarray(np.transpose(sinograms, (1, 0, 2))).astype(
        np.float32
    )
    r = np.zeros((B, D), dtype=np.float32)
    Rs = np.zeros((A, B, D), dtype=np.float32)
    fp = np.empty((B, D), dtype=np.float32)
    res = np.empty((B, D), dtype=np.float32)
    g = np.empty((B, D), dtype=np.float32)
    inv = np.float32(1.0 / 256.0)
    for _ in range(ITERS):
        for a in range(A):
            np.dot(r, C[a], out=fp)
            np.subtract(sino[a], fp, out=res)
            Rs[a] += res
            np.take(res, idx0[a], axis=1, out=g)
            g *= inv
            r += g
    Rs *= inv
    return Rs


def _pack_drs_all(Rs):
    """(NCORES*128, A*KT*2*BPC) fp16 concat of per-core hi+lo lhsT packs."""
    if "drs_perm" not in _cache:
        _cache["drs_perm"] = (
            np.arange(A * 2 * B * D)
            .reshape(A, 2, NCORES, BPC, KT, 128)  # (a, h, c, b, kt, p)
            .transpose(2, 5, 0, 4, 1, 3)          # (c, p, a, kt, h, b)
            .ravel()
        )
    DRs = Rs.copy()
    DRs[:, :, 1:] -= Rs[:, :, :-1]
    hi = DRs.astype(np.float16)
    lo = (DRs - hi.astype(np.float32)).astype(np.float16)
    sl = np.stack([hi, lo], axis=1)  # (A, h, B, D)
    flat = sl.view(np.uint16).ravel()[_cache["drs_perm"]].view(np.float16)
    return flat.reshape(NCORES * 128, A * KT * 2 * BPC)


# ---------------------------------------------------------------- device
def _build_nc(signs):
    nc = bass.Bass()
    f16 = mybir.dt.float16
    f32 = mybir.dt.float32
    thr_d = nc.declare_dram_parameter("thr", [128, NT * H], f16, isOutput=False)
    cg_d = nc.declare_dram_parameter("cg", [128, A * 4 * 256], f16, isOutput=False)
    diff_d = nc.declare_dram_parameter("diff", [128, 2 * 256], f32, isOutput=False)
    sino_d = nc.declare_dram_parameter(
        "sino", [128, A * 2 * BPC], f32, isOutput=False
    )
    out_d = nc.declare_dram_parameter("out", [BPC, H * W], f16, isOutput=True)

    from contextlib import ExitStack

    with ExitStack() as stack:
        ec = stack.enter_context
        thr_sb = ec(nc.sbuf_tensor([128, NT * H], f16))
        sino_sb = ec(nc.sbuf_tensor([128, A * 2 * BPC], f32))
        diff_sb = ec(nc.sbuf_tensor([128, 2 * 128 * 2], f32))  # [dt, kt*128+k]
        cg_sb = ec(nc.sbuf_tensor([128, 3 * 4 * 256], f16))    # [buf, blk, f]
        rs_sb = ec(nc.sbuf_tensor([128, A * 2 * BPC], f32))    # [a, dt, b]
        rt16 = ec(nc.sbuf_tensor([128, 8], f16))               # [kt, h, b]
        r32 = ec(nc.sbuf_tensor([128, 4], f32))                # [kt, b]
        tmp32 = ec(nc.sbuf_tensor([128, 4], f32))              # [dt, b]
        tmp32b = ec(nc.sbuf_tensor([128, 4], f32))             # [kt, b]
        res16 = ec(nc.sbuf_tensor([128, 4], f16))              # [dt, b]
        drs_sb = ec(nc.sbuf_tensor([128, NT * 2 * BPC], f16))  # [a, kt, h, b]
        jfwd = ec(nc.sbuf_tensor([128, 2 * CPIX], f16))
        jrev = ec(nc.sbuf_tensor([128, 2 * CPIX], f16))
        s_sb = ec(nc.sbuf_tensor([128, 2 * CPIX], f16))
        out_sb = ec(nc.sbuf_tensor([BPC, 2 * CPIX], f16))
        psum_bp = ec(nc.psum_tensor([BPC, CPIX], f32))
        psum_fp = ec(nc.psum_tensor([128, 4], f32))            # [jt(=dt), b]
        psum_g = ec(nc.psum_tensor([128, 4], f32))             # [kt, b]
        psum_d = ec(nc.psum_tensor([128, 4], f32))             # [kt, b]
        sem_thr = ec(nc.semaphore())
        sem_sino = ec(nc.semaphore())
        sem_diff = ec(nc.semaphore())
        sem_cg = [ec(nc.semaphore(name=f"sem_cg{i}")) for i in range(3)]
        sem_fp = ec(nc.semaphore())
        sem_res = ec(nc.semaphore())
        sem_gat = ec(nc.semaphore())
        sem_rup = ec(nc.semaphore())
        sem_dmm = ec(nc.semaphore())
        sem_drs = ec(nc.semaphore())
        sem_jch = ec(nc.semaphore())
        sem_s = ec(nc.semaphore())
        sem_sc = ec(nc.semaphore())
        sem_evac = ec(nc.semaphore())
        sem_dout = ec(nc.semaphore())
        sem_vv = ec(nc.semaphore())
        sem_init = ec(nc.semaphore())
        block = ec(nc.Block())
        cg_v = cg_sb[:, :].rearrange("p (u blk f) -> p u blk f", blk=4, f=256)
        rt_v = rt16[:, :].rearrange("p (kt h b) -> p kt h b", h=2, b=BPC)
        r32_v = r32[:, :].rearrange("p (kt b) -> p kt b", b=BPC)
        sino_v = sino_sb[:, :].rearrange("p (a dt b) -> p a dt b", dt=2, b=BPC)
        rs_v = rs_sb[:, :].rearrange("p (a dt b) -> p a dt b", dt=2, b=BPC)
        res_v = res16[:, :].rearrange("p (dt b) -> p dt b", b=BPC)
        diff_v = diff_sb[:, :].rearrange("p (dt f) -> p dt f", f=256)
        drs_v = drs_sb[:, :].rearrange(
            "p (a kt h b) -> p a kt h b", kt=KT, h=2, b=BPC
        )

        @block.scalar
        def _(scalar):
            scalar.dma_start(out=sino_sb[:, :], in_=sino_d[:, :]).then_inc(
                sem_sino, 16
            )
            scalar.dma_start(out=diff_sb[:, :], in_=diff_d[:, :]).then_inc(
                sem_diff, 16
            )
            scalar.dma_start(out=thr_sb[:, :], in_=thr_d[:, :]).then_inc(
                sem_thr, 16
            )
            # backprojection psum evacuation
            for c in range(NCH):
                scalar.wait_ge(sem_sc, NT * (c + 1))
                if c >= 2:
                    scalar.wait_ge(sem_dout, 16 * c)
                src = psum_bp[:, :].rearrange("b (jj ii) -> b jj ii", ii=H)
                dst = out_sb[:, (c % 2) * CPIX:(c % 2 + 1) * CPIX].rearrange(
                    "b (ii jj) -> b jj ii", jj=JC
                )
                scalar.copy(out=dst, in_=src).then_inc(sem_evac, 1)

        @block.sync
        def _(sync):
            # stream cg packs, 3-deep ring
            for t in range(NSTEP):
                a = t % A
                if t >= 3:
                    sync.wait_ge(sem_gat, t - 2)  # step t-3 PE-consumed
                sync.dma_start(
                    out=cg_sb[:, (t % 3) * 1024:(t % 3 + 1) * 1024],
                    in_=cg_d[:, a * 1024:(a + 1) * 1024],
                ).then_inc(sem_cg[t % 3], 16)
            for c in range(NCH):
                sync.wait_ge(sem_evac, c + 1)
                src = out_sb[:, (c % 2) * CPIX:(c % 2 + 1) * CPIX].rearrange(
                    "b (i jj) -> b i jj", jj=JC
                )
                out_v = out_d[:, :].rearrange("b (i j) -> b i j", j=W)
                sync.dma_start(
                    out=out_v[:, :, c * JC:(c + 1) * JC], in_=src
                ).then_inc(sem_dout, 16)

        @block.gpsimd
        def _(gpsimd):
            gpsimd.memset(rt16[:, :], 0).then_inc(sem_init, 1)
            gpsimd.memset(r32[:, :], 0).then_inc(sem_init, 1)
            gpsimd.memset(rs_sb[:, :], 0).then_inc(sem_init, 1)
            for c in range(NCH):
                if c >= 2:
                    gpsimd.wait_ge(sem_s, NT * (c - 1))
                buf = (c % 2) * CPIX
                gpsimd.iota(
                    jfwd[:, buf:buf + CPIX].rearrange("p (jj ii) -> p jj ii", ii=H),
                    [[1, JC], [0, H]],
                    base=c * JC,
                    channel_multiplier=0,
                    allow_small_or_imprecise_dtypes=True,
                )
                gpsimd.iota(
                    jrev[:, buf:buf + CPIX].rearrange("p (jj ii) -> p jj ii", ii=H),
                    [[-1, JC], [0, H]],
                    base=(W - 1) - c * JC,
                    channel_multiplier=0,
                    allow_small_or_imprecise_dtypes=True,
                ).then_inc(sem_jch, 1)

        @block.tensor
        def _(tensor):
            # ---- recurrence ----
            for t in range(NSTEP):
                tensor.wait_ge(sem_cg[t % 3], 16 * (t // 3 + 1))
                if t == 0:
                    tensor.wait_ge(sem_init, 3)
                else:
                    tensor.wait_ge(sem_vv, 7 * t)  # r-update of step t-1
                for jt in range(2):
                    for kt in range(2):
                        for h in range(2):
                            mm = tensor.matmul(
                                psum_fp[:, jt * BPC:(jt + 1) * BPC],
                                cg_v[:, t % 3, kt, jt * 128:(jt + 1) * 128],
                                rt_v[:, kt, h, :],
                                start=(kt == 0 and h == 0),
                                stop=(kt == 1 and h == 1),
                            )
                mm.then_inc(sem_fp, 1)
                # -- gather --
                tensor.wait_ge(sem_vv, 7 * t + 3)  # res16 ready
                for kt in range(2):
                    for dt in range(2):
                        mm = tensor.matmul(
                            psum_g[:, kt * BPC:(kt + 1) * BPC],
                            cg_v[:, t % 3, 2 + dt, kt * 128:(kt + 1) * 128],
                            res_v[:, dt, :],
                            start=(dt == 0),
                            stop=(dt == 1),
                        )
                mm.then_inc(sem_gat, 1)
            # ---- DRs ----
            tensor.wait_ge(sem_diff, 16)
            tensor.wait_ge(sem_vv, 7 * NSTEP)  # recurrence VE fully done
            for a in range(A):
                if a >= 1:
                    tensor.wait_ge(sem_vv, 7 * NSTEP + 3 * a)  # split a-1 done
                for kt in range(2):
                    for dt in range(2):
                        mm = tensor.matmul(
                            psum_d[:, kt * BPC:(kt + 1) * BPC],
                            diff_v[:, dt, kt * 128:(kt + 1) * 128],
                            rs_v[:, a, dt, :],
                            start=(dt == 0),
                            stop=(dt == 1),
                        )
                mm.then_inc(sem_dmm, 1)
            # ---- backprojection ----
            gt = 0
            for c in range(NCH):
                for t in range(NT):
                    a, kt = divmod(t, KT)
                    tensor.wait_ge(sem_s, gt + 1)
                    if t == 0:
                        if c == 0:
                            tensor.wait_ge(sem_vv, 7 * NSTEP + 3 * A)  # drs done
                        else:
                            tensor.wait_ge(sem_evac, c)
                    sb = (gt % 2) * CPIX
                    for h in range(2):
                        lhsT = drs_v[:, a, kt, h, :]
                        for q in range(QN):
                            mm = tensor.matmul(
                                psum_bp[:, q * NMM:(q + 1) * NMM],
                                lhsT,
                                s_sb[:, sb + q * NMM:sb + (q + 1) * NMM],
                                start=(t == 0 and h == 0),
                                stop=(t == NT - 1 and h == 1),
                            )
                    mm.then_inc(sem_sc, 1)
                    gt += 1

        @block.vector
        def _(vector):
            # ---- recurrence partner (VE fully serialized via sem_vv) ----
            vv = [0]

            def step(ins):
                ins.then_inc(sem_vv, 1)
                vv[0] += 1
                vector.wait_ge(sem_vv, vv[0])

            vector.wait_ge(sem_sino, 16)
            for t in range(NSTEP):
                a = t % A
                vector.wait_ge(sem_fp, t + 1)
                step(vector.tensor_tensor(
                    tmp32[:, :],
                    sino_sb[:, a * 4:(a + 1) * 4],
                    psum_fp[:, :],
                    mybir.AluOpType.subtract,
                ))
                step(vector.tensor_tensor(
                    rs_sb[:, a * 4:(a + 1) * 4],
                    rs_sb[:, a * 4:(a + 1) * 4],
                    tmp32[:, :],
                    mybir.AluOpType.add,
                ))
                step(vector.tensor_copy(res16[:, :], tmp32[:, :]))
                vector.wait_ge(sem_gat, t + 1)
                step(vector.tensor_scalar(
                    tmp32b[:, :], psum_g[:, :], INV, None, mybir.AluOpType.mult
                ))
                step(vector.tensor_tensor(
                    r32[:, :], r32[:, :], tmp32b[:, :], mybir.AluOpType.add
                ))
                step(vector.tensor_copy(rt_v[:, :, 0, :], r32_v[:, :, :]))
                step(vector.tensor_tensor(
                    rt_v[:, :, 1, :],
                    r32_v[:, :, :],
                    rt_v[:, :, 0, :],
                    mybir.AluOpType.subtract,
                ))
            # ---- DRs splits ----
            for a in range(A):
                vector.wait_ge(sem_dmm, a + 1)
                step(vector.tensor_scalar(
                    tmp32b[:, :], psum_d[:, :], INV, None, mybir.AluOpType.mult
                ))
                step(vector.tensor_copy(
                    drs_v[:, a, :, 0, :],
                    tmp32b[:, :].rearrange("p (kt b) -> p kt b", b=BPC),
                ))
                step(vector.tensor_tensor(
                    drs_v[:, a, :, 1, :],
                    tmp32b[:, :].rearrange("p (kt b) -> p kt b", b=BPC),
                    drs_v[:, a, :, 0, :],
                    mybir.AluOpType.subtract,
                ))
            # ---- backprojection S builds ----
            gt = 0
            for c in range(NCH):
                vector.wait_ge(sem_jch, c + 1)
                if c == 0:
                    vector.wait_ge(sem_thr, 16)
                for t in range(NT):
                    a, kt = divmod(t, KT)
                    if gt >= 2:
                        vector.wait_ge(sem_sc, gt - 1)
                    jsrc = jfwd if signs[a] else jrev
                    buf = (c % 2) * CPIX
                    in0 = jsrc[:, buf:buf + CPIX].rearrange(
                        "p (jj ii) -> p jj ii", ii=H
                    )
                    base = (a * KT + kt) * H
                    in1 = (
                        thr_sb[:, base:base + H]
                        .unsqueeze(1)
                        .broadcast_to([128, JC, H])
                    )
                    sb = (gt % 2) * CPIX
                    outp = s_sb[:, sb:sb + CPIX].rearrange(
                        "p (jj ii) -> p jj ii", ii=H
                    )
                    vector.tensor_tensor(
                        outp, in0, in1, mybir.AluOpType.is_ge
                    ).then_inc(sem_s, 1)
                    gt += 1

    return nc


# ---------------------------------------------------------------- runner

def _make_runner():
    """Build nc + cached jitted shard_map callable + device-resident thr."""
    if "runner" in _cache:
        return _cache["runner"]

    import jax
    import jax.numpy as jnp
    from jax.experimental.shard_map import shard_map
    from jax.sharding import Mesh, NamedSharding, PartitionSpec

    from concourse.bass2jax import (
        _bass_exec_p,
        install_neuronx_cc_hook,
        partition_id_tensor,
    )

    install_neuronx_cc_hook()

    _, signs = _geometry()
    nc = _build_nc(signs)

    partition_name = nc.partition_id_tensor.name if nc.partition_id_tensor else None
    in_names, out_names, out_avals, zero_shapes = [], [], [], []
    for alloc in nc.m.functions[0].allocations:
        if not isinstance(alloc, mybir.MemoryLocationSet):
            continue
        name = alloc.memorylocations[0].name
        if alloc.kind == "ExternalInput":
            if name != partition_name:
                in_names.append(name)
        elif alloc.kind == "ExternalOutput":
            out_names.append(name)
            shape = tuple(alloc.tensor_shape)
            dtype = mybir.dt.np(alloc.dtype)
            out_avals.append(jax.core.ShapedArray(shape, dtype))
            zero_shapes.append((shape, dtype))
    n_params = len(in_names)
    n_outs = len(out_avals)
    all_in_names = in_names + out_names
    if partition_name is not None:
        all_in_names = all_in_names + [partition_name]

    def _body(*args):
        operands = list(args)
        if partition_name is not None:
            operands.append(partition_id_tensor())
        outs = _bass_exec_p.bind(
            *operands,
            out_avals=tuple(out_avals),
            in_names=tuple(all_in_names),
            out_names=tuple(out_names),
            lowering_input_output_aliases=(),
            sim_require_finite=True,
            sim_require_nnan=True,
            nc=nc,
        )
        return tuple(outs)

    devices = jax.devices()[:NCORES]
    mesh = Mesh(np.asarray(devices), ("core",))
    spec = PartitionSpec("core")
    sharded = jax.jit(
        shard_map(
            _body,
            mesh=mesh,
            in_specs=(spec,) * (n_params + n_outs),
            out_specs=(spec,) * n_outs,
            check_rep=False,
        ),
        donate_argnums=tuple(range(n_params, n_params + n_outs)),
        keep_unused=True,
    )

    def zeros_maker():
        return tuple(
            jnp.zeros((NCORES * s[0], *s[1:]), d) for (s, d) in zero_shapes
        )

    zeros_jit = jax.jit(
        zeros_maker,
        out_shardings=tuple(
            NamedSharding(mesh, spec) for _ in zero_shapes
        ),
    )

    # constant tables (geometry), device-resident once
    sh = NamedSharding(mesh, spec)

    def _replicate(arr):
        g = np.broadcast_to(arr[None], (NCORES, *arr.shape)).reshape(
            NCORES * arr.shape[0], *arr.shape[1:]
        )
        d = jax.device_put(np.ascontiguousarray(g), sh)
        jax.block_until_ready(d)
        return d

    thr_dev = _replicate(_thresholds())

    C, idx0 = _recurrence_consts()
    cg = np.zeros((128, A, 4, 256), np.float16)
    for a in range(A):
        for kt in range(2):
            cg[:, a, kt, :] = C[a, kt * 128:(kt + 1) * 128, :]  # [p, j]
        for dt in range(2):
            cg[:, a, 2 + dt, :] = (
                idx0[a][None, :] == (dt * 128 + np.arange(128))[:, None]
            ).astype(np.float16)  # [p, k]
    cg_dev = _replicate(np.ascontiguousarray(cg.reshape(128, A * 4 * 256)))

    diff = np.zeros((128, 2, 256), np.float32)
    for dt in range(2):
        dvec = dt * 128 + np.arange(128)
        diff[:, dt, :] = (dvec[:, None] == np.arange(256)[None, :]).astype(
            np.float32
        ) - (dvec[:, None] == (np.arange(256) - 1)[None, :]).astype(np.float32)
    diff_dev = _replicate(np.ascontiguousarray(diff.reshape(128, 512)))

    # per-call sinogram permutation: sp[c*128+p, (a, dt, b)] = sino[2c+b, a, dt*128+p]
    src = (
        np.arange(B * A * D)
        .reshape(NCORES, BPC, A, 2, 128)   # (c, b, a, dt, p)
        .transpose(0, 4, 2, 3, 1)          # (c, p, a, dt, b)
        .ravel()
    )
    runner = {
        "sharded": sharded,
        "zeros_jit": zeros_jit,
        "thr_dev": thr_dev,
        "cg_dev": cg_dev,
        "diff_dev": diff_dev,
        "sino_perm": src,
        "in_names": in_names,
        "out_names": out_names,
    }
    _cache["runner"] = runner
    return runner


def kernel(sinograms):
    import jax

    sinograms = np.asarray(sinograms, dtype=np.float32)
    r = _make_runner()
    sp = (
        sinograms.ravel()[r["sino_perm"]]
        .reshape(NCORES * 128, A * 2 * BPC)
    )
    arg_by_name = {
        "thr": r["thr_dev"],
        "cg": r["cg_dev"],
        "diff": r["diff_dev"],
        "sino": sp,
    }
    args = [arg_by_name[n] for n in r["in_names"]]
    outs = r["zeros_jit"]()
    out_arrs = r["sharded"](*args, *outs)
    out_np = np.asarray(out_arrs[0])  # (NCORES*BPC, H*W) == (B, H*W)

    img = out_np.astype(np.float32).reshape(B, H, W)
    np.clip(img, 0.0, img.max(), out=img)
    return img
